# revision 14
# baseline (speedup 1.0000x reference)
"""Trainium2 Bass kernel for AttentionLayerWithMask (ragged prefix-mask attention).

Problem: B=1024, S=200, D=O=512.
  sqlen = mask.sum(1); query = proj_q(x[b, sqlen-1]); keys/values = x[b, :sqlen-1]
  out = tanh(attn @ V)

Algebraic rewrite (exact, up to fp reassociation):
  scores[b,s] = (Wk^T q[b]) . x[b,s]  (+ const in s -> softmax-invariant, dropped)
  out[b]      = tanh(Wv (sum_s attn[b,s] x[b,s]) + bv)

Token-packed layout: only the ~50% VALID tokens are shipped (mean sqlen ~101
of 200). Host sorts batches by length, round-robins them over the 8 cores,
cuts each batch's valid keys into <=T_PIECE-token pieces and packs pieces
into a [128 partitions x NCOL columns] grid (columns of 128 tokens each).
Each "pass" is a column range in which every partition holds tokens of ONE
batch (bid[pass, partition]).

Engine assignment (measured rates):
  VectorE : per-column fused dot  scalar_tensor_tensor(fp16 in, bf16 out,
            f32 accum) -- the accumulator taps the f32 products so fp16
            input precision is preserved (bf16 x fails the 2e-2 gate).
            The dot is VectorE's hard floor (~820ns/col, no DVE fast mode
            exists for fused two-tensor reduce), so a slice of columns is
            offloaded: product on VectorE-TT (~410ns) or GpSimd-TT, with
            the per-column sum done by ScalarE activation(accum_out).
  ScalarE : exp (+accum for z), PSUM->SBUF copies, offloaded column sums.
  GpSimd  : scatter-weight build PG[s,c,b] = onehot[s,b]*p[s,c] as one
            broadcast tensor_tensor per column-group + offloaded products.
  TensorE : ctx[b,:] += sum_s PG[s,c,b]*x[s,c,:]  accumulated in PSUM f32
            (mixed bf16 lhsT x fp16 rhs, verified on hw); per-pass query
            gather; z scatter; projections.

Sharding: pure data parallel, batch 1024 -> 8 cores x 128 partitions.
"""

import numpy as np

B, S, D, O = 1024, 200, 512, 512
NCORES = 8
P = 128                  # batches (partitions) per core
NK = D // 128            # 4 contraction chunks of 128
T_PIECE = 8              # max tokens per piece
G = 8                    # columns per DMA/compute group
N_VSTT = 4               # cols/group: fused dot on VectorE
N_VTT = 4                # cols/group: VectorE product + ScalarE sum
NEG = -1e30              # remaining cols/group: GpSimd product + ScalarE sum

_cache = {}


# ----------------------------------------------------------------------
# packing plan (host, from the actual mask)
# ----------------------------------------------------------------------

def _plan(sqlen):
    lens = sqlen.astype(np.int64) - 1                  # valid key counts >= 1
    order = np.argsort(-lens, kind="stable")           # global desc
    core_batches = [order[k::NCORES] for k in range(NCORES)]  # global idx, local order

    core_pieces = []
    for k in range(NCORES):
        pieces = []
        for lb in range(P):
            L = int(lens[core_batches[k][lb]])
            s = 0
            while s < L:
                pl = min(T_PIECE, L - s)
                pieces.append((lb, s, pl))
                s += pl
        pieces.sort(key=lambda t: -t[2])
        core_pieces.append(pieces)

    npass = max((len(p) + P - 1) // P for p in core_pieces)
    W = []
    for j in range(npass):
        w = 1
        for p in core_pieces:
            if j * P < len(p):
                w = max(w, p[j * P][2])
        W.append(w)
    col0 = [0]
    for w in W:
        col0.append(col0[-1] + w)
    ncol = col0[-1]
    return core_batches, core_pieces, npass, tuple(W), col0, ncol


def _groups(npass, W, col0):
    """Static group structure: per pass, list of (abs col, width<=G)."""
    out = []
    for j in range(npass):
        gs, c = [], col0[j]
        while c < col0[j] + W[j]:
            gw = min(G, col0[j] + W[j] - c)
            gs.append((c, gw))
            c += gw
        out.append(gs)
    return out


# ----------------------------------------------------------------------
# device kernel
# ----------------------------------------------------------------------

def _build_nc(npass, W, col0, ncol):
    from contextlib import ExitStack

    import concourse.bass as bass
    import concourse.tile as tile
    from concourse import bacc, mybir
    from concourse.masks import make_identity

    f32 = mybir.dt.float32
    bf16 = mybir.dt.bfloat16
    fp16 = mybir.dt.float16
    AF = mybir.ActivationFunctionType
    ALU = mybir.AluOpType

    groups = _groups(npass, W, col0)
    ncols_total = sum(W)

    nc = bacc.Bacc("TRN2", target_bir_lowering=False, debug=False, num_devices=NCORES)

    x_d = nc.dram_tensor("x", [P, ncol, D], fp16, kind="ExternalInput").ap()
    maskc_d = nc.dram_tensor("maskc", [P, ncol], f32, kind="ExternalInput").ap()
    bidc_d = nc.dram_tensor("bidc", [P, npass], f32, kind="ExternalInput").ap()
    lastT_d = nc.dram_tensor("lastT", [D, P], fp16, kind="ExternalInput").ap()
    wqT_d = nc.dram_tensor("wqT", [D, O], fp16, kind="ExternalInput").ap()
    wk_d = nc.dram_tensor("wk", [O, D], fp16, kind="ExternalInput").ap()
    wvT_d = nc.dram_tensor("wvT", [D, O], fp16, kind="ExternalInput").ap()
    bq_d = nc.dram_tensor("bq", [NK, 128, 1], f32, kind="ExternalInput").ap()
    bv_d = nc.dram_tensor("bv", [1, O], f32, kind="ExternalInput").ap()
    iota_d = nc.dram_tensor("iotaf", [128, 128], f32, kind="ExternalInput").ap()
    out_d = nc.dram_tensor("out", [P, O], f32, kind="ExternalOutput").ap()

    with tile.TileContext(nc) as tc:
        with ExitStack() as ctx:
            consts = ctx.enter_context(tc.tile_pool(name="consts", bufs=1))
            xg_pool = ctx.enter_context(tc.tile_pool(name="xg", bufs=5))
            prod_pool = ctx.enter_context(tc.tile_pool(name="prod", bufs=4))
            pg_pool = ctx.enter_context(tc.tile_pool(name="pg", bufs=3))
            small = ctx.enter_context(tc.tile_pool(name="small", bufs=4))
            pass_pool = ctx.enter_context(tc.tile_pool(name="pass", bufs=1))
            ps_small = ctx.enter_context(tc.tile_pool(name="psS", bufs=1, space="PSUM"))
            ps_big = ctx.enter_context(tc.tile_pool(name="psB", bufs=2, space="PSUM"))
            ps_ctx = ctx.enter_context(tc.tile_pool(name="psC", bufs=1, space="PSUM"))
            ps_z = ctx.enter_context(tc.tile_pool(name="psZ", bufs=1, space="PSUM"))

            # ---------- constants ----------
            wqT_sb, wk_sb, wvT_sb, lastT_sb, bq_sb = [], [], [], [], []
            for k in range(NK):
                t = consts.tile([128, O], fp16, tag=f"wqT{k}")
                nc.sync.dma_start(t, wqT_d[k * 128:(k + 1) * 128, :])
                wqT_sb.append(t)
                t = consts.tile([128, D], fp16, tag=f"wk{k}")
                nc.sync.dma_start(t, wk_d[k * 128:(k + 1) * 128, :])
                wk_sb.append(t)
                t = consts.tile([128, O], fp16, tag=f"wvT{k}")
                nc.sync.dma_start(t, wvT_d[k * 128:(k + 1) * 128, :])
                wvT_sb.append(t)
                t = consts.tile([128, P], fp16, tag=f"lastT{k}")
                nc.sync.dma_start(t, lastT_d[k * 128:(k + 1) * 128, :])
                lastT_sb.append(t)
                t = consts.tile([128, 1], f32, tag=f"bq{k}")
                nc.sync.dma_start(t, bq_d[k])
                bq_sb.append(t)
            bv_sb = consts.tile([1, O], f32, tag="bv")
            nc.sync.dma_start(bv_sb, bv_d)
            iota_sb = consts.tile([128, 128], f32, tag="iota")
            nc.sync.dma_start(iota_sb, iota_d)
            maskc_sb = consts.tile([P, ncol], f32, tag="maskc")
            nc.sync.dma_start(maskc_sb, maskc_d)
            bidc_sb = consts.tile([P, npass], f32, tag="bidc")
            nc.sync.dma_start(bidc_sb, bidc_d)
            ones_sb = consts.tile([1, 128], f32, tag="ones")
            nc.vector.memset(ones_sb, 1.0)
            ident_h = consts.tile([128, 128], fp16, tag="identh")
            make_identity(nc, ident_h)
            ident_b = consts.tile([128, 128], bf16, tag="identb")
            make_identity(nc, ident_b)

            # ---------- QT[o,b] = Wq @ last + bq  (fp16 matmuls) ----------
            qt_sb = []
            for om in range(NK):
                pq = ps_small.tile([128, P], f32, tag="tr", name=f"pq{om}")
                for kd in range(NK):
                    nc.tensor.matmul(
                        pq, lhsT=wqT_sb[kd][:, om * 128:(om + 1) * 128],
                        rhs=lastT_sb[kd], start=(kd == 0), stop=(kd == NK - 1))
                qt = consts.tile([128, P], fp16, tag=f"qt{om}")
                nc.scalar.activation(qt, pq, AF.Identity, bias=bq_sb[om], scale=1.0)
                qt_sb.append(qt)

            # ---------- QK[b,d] = q^T Wk  (folded query, batch-major) ----------
            pqk = ps_big.tile([P, D], f32, tag="big", name="pqk")
            for ko in range(NK):
                nc.tensor.matmul(pqk, lhsT=qt_sb[ko], rhs=wk_sb[ko],
                                 start=(ko == 0), stop=(ko == NK - 1))
            qkb_h = consts.tile([P, D], fp16, tag="qkb")
            nc.scalar.copy(qkb_h, pqk)

            # ---------- per-pass gather state, built upfront ----------
            ctx_ps = ps_ctx.tile([P, D], f32, tag="ctx")
            z_ps = ps_z.tile([P, 1], f32, tag="z")
            onehots, qkps = [], []
            for j in range(npass):
                onehot = pass_pool.tile([P, 128], bf16, tag=f"oh{j}")
                nc.vector.tensor_scalar(
                    out=onehot, in0=iota_sb, scalar1=bidc_sb[:, j:j + 1],
                    scalar2=None, op0=ALU.is_equal)
                ptr = ps_small.tile([128, 128], bf16, tag="trb", name=f"ohT{j}")
                nc.tensor.transpose(ptr, onehot, ident_b)
                onehotT = pass_pool.tile([128, P], bf16, tag="ohT",
                                         name=f"ohTs{j}")
                nc.scalar.copy(onehotT, ptr)
                # gather per-partition folded queries: qkp[p,:] = qkb[bid[p],:]
                qk_ps = ps_big.tile([128, D], f32, tag="big", name=f"qkg{j}")
                nc.tensor.matmul(qk_ps, lhsT=onehotT, rhs=qkb_h,
                                 start=True, stop=True)
                qkp = pass_pool.tile([128, D], fp16, tag=f"qkp{j}")
                nc.scalar.copy(qkp, qk_ps)
                onehots.append(onehot)
                qkps.append(qkp)

            # ---------- main loop: passes x column-groups ----------
            col_seen = 0
            for j in range(npass):
                onehot, qkp = onehots[j], qkps[j]
                single = len(groups[j]) == 1
                if not single:
                    zpass = small.tile([P, 1], f32, tag="zp", name=f"zp{j}")
                    nc.vector.memset(zpass, 0.0)

                for (c0, gw) in groups[j]:
                    xg = xg_pool.tile([P, gw, D], fp16, tag=f"xg{gw}",
                                      name=f"xg{c0}")
                    nc.sync.dma_start(xg, x_d[:, c0:c0 + gw, :])

                    sc = small.tile([P, gw], f32, tag=f"sc{gw}", name=f"sc{c0}")
                    n_vstt = min(N_VSTT, gw)
                    n_vtt = min(N_VTT, gw - n_vstt)
                    # offloaded columns first: their ScalarE sums overlap the
                    # VectorE fused dots that follow
                    for jj in range(n_vstt, gw):
                        # product on V-TT / GpSimd-TT, column sum on ScalarE
                        prod = prod_pool.tile([P, D], fp16, tag="prodh",
                                              name=f"ph{c0}_{jj}")
                        eng = nc.vector if jj < n_vstt + n_vtt else nc.gpsimd
                        eng.tensor_tensor(out=prod, in0=xg[:, jj, :], in1=qkp,
                                          op=ALU.mult)
                        junk = prod_pool.tile([P, D], fp16, tag="junk",
                                              name=f"jk{c0}_{jj}")
                        nc.scalar.activation(junk, prod, AF.Copy,
                                             accum_out=sc[:, jj:jj + 1])
                    for jj in range(n_vstt):
                        prod = prod_pool.tile([P, D], bf16, tag="prod",
                                              name=f"pr{c0}_{jj}")
                        nc.vector.scalar_tensor_tensor(
                            out=prod, in0=xg[:, jj, :], scalar=1.0, in1=qkp,
                            op0=ALU.mult, op1=ALU.mult,
                            accum_out=sc[:, jj:jj + 1])
                    nc.vector.tensor_add(sc, sc, maskc_sb[:, c0:c0 + gw])

                    pg = small.tile([P, gw], f32, tag=f"pe{gw}", name=f"pe{c0}")
                    zg = small.tile([P, 1], f32, tag="zg", name=f"zg{c0}")
                    nc.scalar.activation(pg, sc, AF.Exp, accum_out=zg)
                    if single:
                        zpass = zg
                    else:
                        nc.vector.tensor_add(zpass, zpass, zg)

                    # PG[s, c, b] = onehot[s, b] * p[s, c]  (one gpsimd op)
                    PGt = pg_pool.tile([P, gw, 128], bf16, tag=f"PG{gw}",
                                       name=f"PG{c0}")
                    nc.gpsimd.tensor_tensor(
                        out=PGt,
                        in0=onehot.unsqueeze(1).to_broadcast([P, gw, 128]),
                        in1=pg.unsqueeze(2).to_broadcast([P, gw, 128]),
                        op=ALU.mult)
                    for jj in range(gw):
                        nc.tensor.matmul(
                            ctx_ps, lhsT=PGt[:, jj, :], rhs=xg[:, jj, :],
                            start=(col_seen == 0),
                            stop=(col_seen == ncols_total - 1),
                            skip_group_check=True)
                        col_seen += 1

                # scatter this pass's z by batch: z[b] += sum_p onehot[p,b]*zpass[p]
                zbf = small.tile([P, 1], bf16, tag="zbf", name=f"zbf{j}")
                nc.vector.tensor_copy(zbf, zpass)
                nc.tensor.matmul(z_ps, lhsT=onehot, rhs=zbf,
                                 start=(j == 0), stop=(j == npass - 1),
                                 skip_group_check=True)

            # ---------- normalize, project, tanh ----------
            zsb = small.tile([P, 1], f32, tag="zsb")
            nc.vector.tensor_copy(zsb, z_ps)
            rz = small.tile([P, 1], f32, tag="rz")
            nc.vector.reciprocal(rz, zsb)
            ctxf = consts.tile([P, D], fp16, tag="ctxf")
            nc.scalar.activation(ctxf, ctx_ps, AF.Copy, scale=rz)

            ctxT_sb = []
            for kd in range(NK):
                ptk = ps_small.tile([128, 128], fp16, tag="trh", name=f"ptk{kd}")
                nc.tensor.transpose(ptk, ctxf[:, kd * 128:(kd + 1) * 128], ident_h)
                t = consts.tile([128, P], fp16, tag=f"ctxT{kd}")
                nc.scalar.copy(t, ptk)
                ctxT_sb.append(t)
            pout = ps_big.tile([P, O], f32, tag="big", name="pout")
            for kd in range(NK):
                nc.tensor.matmul(pout, lhsT=ctxT_sb[kd], rhs=wvT_sb[kd],
                                 start=(kd == 0), stop=False,
                                 skip_group_check=True)
            nc.tensor.matmul(pout, lhsT=ones_sb, rhs=bv_sb, start=False,
                             stop=True, skip_group_check=True)
            outt = consts.tile([P, O], f32, tag="outt")
            nc.scalar.activation(outt, pout, AF.Tanh)
            nc.sync.dma_start(out_d, outt)

    nc.compile()
    return nc


# ----------------------------------------------------------------------
# host prep
# ----------------------------------------------------------------------

def _host_prep(input, mask, Wq_w, Wq_b, Wk_w, Wk_b, Wv_w, Wv_b):
    fp16 = np.float16

    input = np.ascontiguousarray(input, dtype=np.float32)
    mask = np.asarray(mask)
    sqlen = mask.astype(np.int64).sum(axis=1)
    core_batches, core_pieces, npass, W, col0, ncol = _plan(sqlen)

    xh = input.astype(fp16)
    last = input[np.arange(B), sqlen - 1]              # [B, D] f32

    wqT = np.ascontiguousarray(np.asarray(Wq_w, np.float32).T).astype(fp16)
    wk = np.ascontiguousarray(np.asarray(Wk_w, np.float32)).astype(fp16)
    wvT = np.ascontiguousarray(np.asarray(Wv_w, np.float32).T).astype(fp16)
    bq = np.ascontiguousarray(np.asarray(Wq_b, np.float32).reshape(NK, 128, 1))
    bv = np.ascontiguousarray(np.asarray(Wv_b, np.float32).reshape(1, O))
    iota = np.ascontiguousarray(
        np.broadcast_to(np.arange(128, dtype=np.float32)[None, :], (128, 128)))
    # Wk_b drops out of softmax (constant shift); Wv_b enters via ones-row matmul.

    in_maps = []
    for k in range(NCORES):
        gidx = core_batches[k]
        src_b = np.zeros((P, ncol), np.int64)
        src_s = np.zeros((P, ncol), np.int64)
        valid = np.zeros((P, ncol), bool)
        bidc = np.full((P, npass), -1.0, np.float32)
        for i, (lb, s, pl) in enumerate(core_pieces[k]):
            j, p = i // P, i % P
            c = col0[j]
            src_b[p, c:c + pl] = gidx[lb]
            src_s[p, c:c + pl] = np.arange(s, s + pl)
            valid[p, c:c + pl] = True
            bidc[p, j] = lb
        xcols = xh[src_b, src_s]                       # [P, ncol, D] fp16
        maskc = np.where(valid, np.float32(0.0), np.float32(NEG))
        lastT = np.ascontiguousarray(last[gidx].T).astype(fp16)
        in_maps.append({
            "x": np.ascontiguousarray(xcols),
            "maskc": np.ascontiguousarray(maskc),
            "bidc": np.ascontiguousarray(bidc),
            "lastT": lastT,
            "wqT": wqT, "wk": wk, "wvT": wvT, "bq": bq, "bv": bv,
            "iotaf": iota,
        })
    plan = (npass, W, tuple(col0), ncol, [np.asarray(g) for g in core_batches])
    return in_maps, plan


def _run(in_maps, plan, trace=False):
    from concourse.bass_utils import run_bass_kernel_spmd
    npass, W, col0, ncol, _ = plan
    key = (npass, W, ncol)
    if key not in _cache:
        _cache[key] = _build_nc(npass, list(W), list(col0), ncol)
    res = run_bass_kernel_spmd(_cache[key], in_maps, list(range(NCORES)),
                               trace=trace)
    return res


def _assemble(res, plan):
    out = np.empty((B, O), np.float32)
    core_batches = plan[4]
    for k in range(NCORES):
        out[core_batches[k]] = res.results[k]["out"]
    return out


def kernel(input, mask, Wq_w, Wq_b, Wk_w, Wk_b, Wv_w, Wv_b):
    in_maps, plan = _host_prep(input, mask, Wq_w, Wq_b, Wk_w, Wk_b, Wv_w, Wv_b)
    res = _run(in_maps, plan, trace=False)
    return _assemble(res, plan)
